# revision 24
# baseline (speedup 1.0000x reference)
"""Multi-head attention (B=8, N=1024, C=768, H=12) for 8 Trainium2 NeuronCores.

Sharding: data-parallel over the batch dim — core b computes batch element b.
Weights are replicated; no collectives.

Per-core plan (all layouts picked so that NO on-device transposes are needed):
  host feeds xT=[C,N] (x[b].T), wqkvT=[C,3C], wprojT=[C,C], bproj=[C].
  1. qT/kT GEMM:  qkT[d, n] = wqkvT_chunk.T @ xT          (d on partitions)
  2. V GEMM:      V[n, dv]  = xT_chunk.T @ wqkvT[:, 2C:]  (natural layout),
                  stored head-strided as V''[n, 12*(64+1)] with a ones column
                  per head (the ones row of V''.T yields the softmax
                  denominator for free during the PV matmul).
  3. Attention per head pair (2 heads packed into the 128-wide PE array by
     row tiling, since head_dim=64):
       S^T[nk, nq]  = kT_chunk.T @ qT          (one matmul per nk chunk)
       expS         = exp(SCALE * S^T)         (ACT engine, PSUM -> SBUF;
                                               max-free softmax: scaled
                                               scores are ~N(0,1), fp32-safe)
       O'[65, nq]  += V''_chunk.T @ expS       (rows 0:64 = unnormalized out^T,
                                               row 64 = sum of exp = denom)
       attnT[c, n]  = O'[0:64] * (1/denom)     (reciprocal + partition-bcast)
  4. proj:        y[n, d2] = attnT_chunk.T @ wprojT + bproj
All matmuls run as float32r (full PE rate at N>=256, ~fp32 precision).
"""

import sys

for _p in ("/opt/trn_rl_repo", "/opt/pypackages"):
    if _p not in sys.path:
        sys.path.append(_p)

import numpy as np

import concourse.bass as bass
import concourse.tile as tile
from concourse import bacc, mybir
from concourse.bass_utils import run_bass_kernel_spmd

B, N, C = 8, 1024, 768
H, HD = 12, 64
SCALE = HD**-0.5
NCORES = 8
KC = C // 128  # 6 contraction chunks over C
NT = N // 128  # 8 chunks over sequence (nk / n-tiles)
NQT = N // 512  # 2 moving-dim tiles over the query sequence
PAIRS = H // 2  # 6 head pairs
F32 = mybir.dt.float32
F32R = mybir.dt.float32r
BF16 = mybir.dt.bfloat16
EXP = mybir.ActivationFunctionType.Exp


def _emit(tc, nc, xT, wqkvT, wprojT, bproj, y, ctx):
    persist = ctx.enter_context(tc.tile_pool(name="persist", bufs=1))
    wqk_pool = ctx.enter_context(tc.tile_pool(name="wqk", bufs=12))
    work = ctx.enter_context(tc.tile_pool(name="work", bufs=3))
    expp = ctx.enter_context(tc.tile_pool(name="expp", bufs=6))
    rdp = ctx.enter_context(tc.tile_pool(name="rdp", bufs=8))
    dram_scr = ctx.enter_context(tc.tile_pool(name="dram_scr", bufs=8, space="DRAM"))
    # 8 PSUM banks total: ps_big = 3 slots x [128,1024] (2 banks each),
    # ps_o = 2 slots x [65,512] (1 bank each).
    ps_big = ctx.enter_context(tc.tile_pool(name="ps_big", bufs=3, space="PSUM"))
    ps_o = ctx.enter_context(tc.tile_pool(name="ps_o", bufs=2, space="PSUM"))

    # ---- persistent loads ----
    # Split into half-tiles alternating issue engines so the transfers spread
    # across DMA rings: phase B needs all of xT+wv (5.25 MB) before its first
    # accumulation completes, and a single ring runs at ~100 GB/s.
    eng = [nc.sync, nc.scalar]
    xTs = []
    for kc in range(KC):
        t = persist.tile([128, N], F32R, tag=f"xT{kc}")
        for h in range(2):
            eng[(2 * kc + h) % 2].dma_start(
                out=t[:, h * 512 : (h + 1) * 512],
                in_=xT[kc * 128 : (kc + 1) * 128, h * 512 : (h + 1) * 512],
            )
        xTs.append(t)
    wvs = []
    for kc in range(KC):
        t = persist.tile([128, C], F32R, tag=f"wv{kc}")
        for h in range(2):
            eng[(2 * kc + h + 1) % 2].dma_start(
                out=t[:, h * 384 : (h + 1) * 384],
                in_=wqkvT[kc * 128 : (kc + 1) * 128, 2 * C + h * 384 : 2 * C + (h + 1) * 384],
            )
        wvs.append(t)
    bpb = persist.tile([128, C], F32, tag="bpb")
    nc.gpsimd.dma_start(
        out=bpb,
        in_=bass.AP(tensor=bproj.tensor, offset=bproj.offset, ap=[[0, 128]] + list(bproj.ap)),
    )

    def emit_wp_loads():
        # wproj is only needed by phase E; load it mid-kernel, off the
        # startup-critical DMA window.
        wps = []
        for kc in range(KC):
            t = persist.tile([128, C], F32R, tag=f"wp{kc}", name=f"wp{kc}")
            for h in range(2):
                eng[(2 * kc + h) % 2].dma_start(
                    out=t[:, h * 384 : (h + 1) * 384],
                    in_=wprojT[kc * 128 : (kc + 1) * 128, h * 384 : (h + 1) * 384],
                )
            wps.append(t)
        return wps

    # ---- phase B: V GEMM (natural layout, head-strided with ones column) ----
    v2s = []
    for nt in range(NT):
        v2s.append(persist.tile([128, H * (HD + 1)], BF16, tag=f"v2{nt}", name=f"v2{nt}"))
    for nt in range(NT):
        psv = ps_big.tile([128, 1024], F32, tag="ps_big")
        for kc in range(KC):
            lhsT = xTs[kc][:, nt * 128 : (nt + 1) * 128]
            nc.tensor.matmul(
                psv[:, 0:512], lhsT, wvs[kc][:, 0:512], start=(kc == 0), stop=(kc == KC - 1)
            )
            nc.tensor.matmul(
                psv[:, 512:768], lhsT, wvs[kc][:, 512:768], start=(kc == 0), stop=(kc == KC - 1)
            )
        v2v = v2s[nt].rearrange("p (h e) -> p h e", e=HD + 1)
        nc.vector.tensor_copy(
            out=v2v[:, :, 0:HD], in_=psv[:, 0:768].rearrange("p (h e) -> p h e", e=HD)
        )
        nc.vector.memset(v2v[:, :, HD : HD + 1], 1.0)

    # ---- phases C (qk GEMM) + D (attention), interleaved per head pair ----
    def emit_qk(j, qT, kT):
        # k first, then q: the k casts overlap the q GEMM, so attention's
        # first S matmul only waits on q's first half-cast.
        for dst, dcol0 in ((kT, C + j * 128), (qT, j * 128)):
            psq = ps_big.tile([128, 1024], F32, tag="ps_big", name="psqk")
            for kc in range(KC):
                w = wqk_pool.tile([128, 128], F32R, tag="wqk")
                nc.sync.dma_start(
                    out=w, in_=wqkvT[kc * 128 : (kc + 1) * 128, dcol0 : dcol0 + 128]
                )
                for h2 in range(NQT):
                    nc.tensor.matmul(
                        psq[:, h2 * 512 : (h2 + 1) * 512],
                        w,
                        xTs[kc][:, h2 * 512 : (h2 + 1) * 512],
                        start=(kc == 0),
                        stop=(kc == KC - 1),
                    )
            for h2 in range(NQT):
                nc.vector.tensor_copy(
                    out=dst[:, h2 * 512 : (h2 + 1) * 512], in_=psq[:, h2 * 512 : (h2 + 1) * 512]
                )

    def emit_attn(j, qT, kT, aT):
        # One [128,1024] S tile per nk chunk: head A in cols 0:512, head B in
        # 512:1024 — a single EXP covers both heads. Chunk-granular skew-2
        # software pipeline streaming continuously across the two nq tiles,
        # so the in-order PE never waits on ACT except at fill/drain.
        SKEW = 3
        steps = []
        oab = {}
        for step in range(NQT * NT + SKEW):
            if step < NQT * NT:
                nq, nkc = divmod(step, NT)
                s = ps_big.tile([128, 1024], F32, tag="ps_big", name="sAB")
                for half, kt0 in ((0, 0), (1, 64)):
                    nc.tensor.matmul(
                        s[:, half * 512 : (half + 1) * 512],
                        kT[kt0 : kt0 + 64, nkc * 128 : (nkc + 1) * 128],
                        qT[kt0 : kt0 + 64, nq * 512 : (nq + 1) * 512],
                        tile_position=(kt0, 0),
                    )
                e = expp.tile([128, 1024], BF16, tag="expp", name="eAB")
                nc.scalar.activation(out=e, in_=s, func=EXP, scale=SCALE)
                steps.append((nq, nkc, e))
            if step >= SKEW:
                nq, nkc, e = steps[step - SKEW]
                if nkc == 0:
                    oab[nq] = (
                        ps_o.tile([HD + 1, 512], F32, tag="ps_o", name="oA"),
                        ps_o.tile([HD + 1, 512], F32, tag="ps_o", name="oB"),
                    )
                oA, oB = oab[nq]
                v2v = v2s[nkc].rearrange("p (h e) -> p h e", e=HD + 1)
                nc.tensor.matmul(
                    oA,
                    v2v[:, 2 * j, :],
                    e[:, 0:512],
                    start=(nkc == 0),
                    stop=(nkc == NT - 1),
                )
                nc.tensor.matmul(
                    oB,
                    v2v[:, 2 * j + 1, :],
                    e[:, 512:1024],
                    start=(nkc == 0),
                    stop=(nkc == NT - 1),
                )
                if nkc == NT - 1:
                    for o, half in ((oA, 0), (oB, 1)):
                        # Drain O' to SBUF at once so the PSUM bank frees for
                        # the next nq tile; kick off the DRAM bounce that
                        # spreads the 512 denominators across 128 partitions.
                        # The reciprocal + multiply are emitted one pair later
                        # (emit_normalize) so their DMA dependencies are long
                        # resolved and never stall the DVE.
                        oc = rdp.tile([HD + 1, 512], F32, tag="oc")
                        nc.vector.tensor_copy(out=oc, in_=o)
                        scr = dram_scr.tile([1, 512], F32, tag="scr")
                        nc.gpsimd.dma_start(out=scr, in_=oc[HD : HD + 1, :])
                        rs = rdp.tile([128, 4], F32, tag="rs")
                        nc.gpsimd.dma_start(
                            out=rs,
                            in_=bass.AP(
                                tensor=scr.tensor, offset=scr.offset, ap=[[4, 128], [1, 4]]
                            ),
                        )
                        pending.append((oc, rs, aT, half, nq))

    def emit_normalize():
        for oc, rs, aT, half, nq in pending:
            rs2 = rdp.tile([128, 4], F32, tag="rs2")
            nc.vector.reciprocal(out=rs2, in_=rs)
            scr2 = dram_scr.tile([1, 512], F32, tag="scr2")
            nc.gpsimd.dma_start(
                out=bass.AP(tensor=scr2.tensor, offset=scr2.offset, ap=[[4, 128], [1, 4]]),
                in_=rs2,
            )
            rb = rdp.tile([64, 512], F32, tag="rb")
            nc.gpsimd.dma_start(
                out=rb,
                in_=bass.AP(tensor=scr2.tensor, offset=scr2.offset, ap=[[0, 64]] + list(scr2.ap[1:])),
            )
            nc.vector.tensor_mul(
                out=aT[half * 64 : half * 64 + 64, nq * 512 : (nq + 1) * 512],
                in0=oc[0:HD, :],
                in1=rb,
            )
        pending.clear()

    aTs = []
    pending = []
    for j in range(PAIRS):
        qT = persist.tile([128, N], BF16, tag=f"qT{j}")
        kT = persist.tile([128, N], BF16, tag=f"kT{j}")
        aT = persist.tile([128, N], F32R, tag=f"aT{j}")
        aTs.append(aT)
        emit_qk(j, qT, kT)
        emit_normalize()  # flush pair j-1 (its bounce DMAs resolved long ago)
        emit_attn(j, qT, kT, aT)
        if j == 1:
            wps = emit_wp_loads()
    emit_normalize()

    # ---- phase E: proj + bias ----
    for nt in range(NT):
        psy = ps_big.tile([128, 1024], F32, tag="ps_big", name="psy")
        for kc in range(KC):
            lhsT = aTs[kc][:, nt * 128 : (nt + 1) * 128]
            nc.tensor.matmul(
                psy[:, 0:512], lhsT, wps[kc][:, 0:512], start=(kc == 0), stop=(kc == KC - 1)
            )
            nc.tensor.matmul(
                psy[:, 512:768], lhsT, wps[kc][:, 512:768], start=(kc == 0), stop=(kc == KC - 1)
            )
        yb = work.tile([128, C], F32, tag="yb")
        nc.vector.tensor_add(out=yb, in0=psy[:, 0:768], in1=bpb)
        for h in range(2):
            eng[h].dma_start(
                out=y[nt * 128 : (nt + 1) * 128, h * 384 : (h + 1) * 384],
                in_=yb[:, h * 384 : (h + 1) * 384],
            )


def build():
    from contextlib import ExitStack

    nc = bacc.Bacc("TRN2", target_bir_lowering=False, debug=False)
    xT = nc.dram_tensor("xT", [C, N], F32R, kind="ExternalInput").ap()
    wqkvT = nc.dram_tensor("wqkvT", [C, 3 * C], F32R, kind="ExternalInput").ap()
    wprojT = nc.dram_tensor("wprojT", [C, C], F32R, kind="ExternalInput").ap()
    bproj = nc.dram_tensor("bproj", [C], F32, kind="ExternalInput").ap()
    y = nc.dram_tensor("y", [N, C], F32, kind="ExternalOutput").ap()
    with tile.TileContext(nc) as tc:
        with ExitStack() as ctx:
            _emit(tc, nc, xT, wqkvT, wprojT, bproj, y, ctx)
    nc.compile()
    return nc


_NC_CACHE = {}


def make_in_maps(x, w_qkv, w_proj, b_proj):
    wqkvT = np.ascontiguousarray(np.asarray(w_qkv).T)
    wprojT = np.ascontiguousarray(np.asarray(w_proj).T)
    b_proj = np.asarray(b_proj)
    return [
        {
            "xT": np.ascontiguousarray(np.asarray(x[b]).T),
            "wqkvT": wqkvT,
            "wprojT": wprojT,
            "bproj": b_proj,
        }
        for b in range(NCORES)
    ]


def kernel(x, w_qkv, w_proj, b_proj, _trace=False, _tmpdir=None):
    if "nc" not in _NC_CACHE:
        _NC_CACHE["nc"] = build()
    nc = _NC_CACHE["nc"]
    in_maps = make_in_maps(x, w_qkv, w_proj, b_proj)
    kwargs = {}
    if _trace:
        kwargs = {"trace": True, "tmpdir": _tmpdir}
    res = run_bass_kernel_spmd(nc, in_maps, core_ids=list(range(NCORES)), **kwargs)
    out = np.stack([res.results[i]["y"] for i in range(NCORES)], axis=0)
    if _trace:
        _NC_CACHE["last_result"] = res
    return out


if __name__ == "__main__":
    rng = np.random.default_rng(0)
    x = rng.standard_normal((B, N, C), dtype=np.float32)
    w_qkv = (rng.standard_normal((3 * C, C), dtype=np.float32) * C**-0.5).astype(np.float32)
    w_proj = (rng.standard_normal((C, C), dtype=np.float32) * C**-0.5).astype(np.float32)
    b_proj = np.zeros(C, dtype=np.float32)
    out = kernel(x, w_qkv, w_proj, b_proj)
    print("out", out.shape, out.dtype, float(np.abs(out).mean()))


# revision 25
# speedup vs baseline: 1.0206x; 1.0206x over previous
"""Multi-head attention (B=8, N=1024, C=768, H=12) for 8 Trainium2 NeuronCores.

Sharding: data-parallel over the batch dim — core b computes batch element b.
Weights are replicated; no collectives.

Per-core plan (all layouts picked so that NO on-device transposes are needed):
  host feeds xT=[C,N] (x[b].T), wqkvT=[C,3C], wprojT=[C,C], bproj=[C].
  1. qT/kT GEMM:  qkT[d, n] = wqkvT_chunk.T @ xT          (d on partitions)
  2. V GEMM:      V[n, dv]  = xT_chunk.T @ wqkvT[:, 2C:]  (natural layout),
                  stored head-strided as V''[n, 12*(64+1)] with a ones column
                  per head (the ones row of V''.T yields the softmax
                  denominator for free during the PV matmul).
  3. Attention per head pair (2 heads packed into the 128-wide PE array by
     row tiling, since head_dim=64):
       S^T[nk, nq]  = kT_chunk.T @ qT          (one matmul per nk chunk)
       expS         = exp(SCALE * S^T)         (ACT engine, PSUM -> SBUF;
                                               max-free softmax: scaled
                                               scores are ~N(0,1), fp32-safe)
       O'[65, nq]  += V''_chunk.T @ expS       (rows 0:64 = unnormalized out^T,
                                               row 64 = sum of exp = denom)
       attnT[c, n]  = O'[0:64] * (1/denom)     (reciprocal + partition-bcast)
  4. proj:        y[n, d2] = attnT_chunk.T @ wprojT + bproj
All matmuls run as float32r (full PE rate at N>=256, ~fp32 precision).
"""

import sys

for _p in ("/opt/trn_rl_repo", "/opt/pypackages"):
    if _p not in sys.path:
        sys.path.append(_p)

import numpy as np

import concourse.bass as bass
import concourse.tile as tile
from concourse import bacc, mybir
from concourse.bass_utils import run_bass_kernel_spmd

B, N, C = 8, 1024, 768
H, HD = 12, 64
SCALE = HD**-0.5
NCORES = 8
KC = C // 128  # 6 contraction chunks over C
NT = N // 128  # 8 chunks over sequence (nk / n-tiles)
NQT = N // 512  # 2 moving-dim tiles over the query sequence
PAIRS = H // 2  # 6 head pairs
F32 = mybir.dt.float32
F32R = mybir.dt.float32r
BF16 = mybir.dt.bfloat16
EXP = mybir.ActivationFunctionType.Exp


def _emit(tc, nc, xT, wqkvT, wprojT, bproj, y, ctx):
    persist = ctx.enter_context(tc.tile_pool(name="persist", bufs=1))
    wqk_pool = ctx.enter_context(tc.tile_pool(name="wqk", bufs=12))
    work = ctx.enter_context(tc.tile_pool(name="work", bufs=3))
    expp = ctx.enter_context(tc.tile_pool(name="expp", bufs=6))
    rdp = ctx.enter_context(tc.tile_pool(name="rdp", bufs=8))
    dram_scr = ctx.enter_context(tc.tile_pool(name="dram_scr", bufs=8, space="DRAM"))
    # 8 PSUM banks total: ps_big = 3 slots x [128,1024] (2 banks each),
    # ps_o = 2 slots x [65,512] (1 bank each).
    ps_big = ctx.enter_context(tc.tile_pool(name="ps_big", bufs=3, space="PSUM"))
    ps_o = ctx.enter_context(tc.tile_pool(name="ps_o", bufs=2, space="PSUM"))

    # ---- persistent loads ----
    # Split into half-tiles alternating issue engines so the transfers spread
    # across DMA rings: phase B needs all of xT+wv (5.25 MB) before its first
    # accumulation completes, and a single ring runs at ~100 GB/s.
    eng = [nc.sync, nc.scalar]
    xTs = []
    wvs = []
    for kc in range(KC):
        tx = persist.tile([128, N], F32R, tag=f"xT{kc}", name=f"xT{kc}")
        tv = persist.tile([128, C], F32R, tag=f"wv{kc}", name=f"wv{kc}")
        for h in range(2):
            eng[(2 * kc + h) % 2].dma_start(
                out=tx[:, h * 512 : (h + 1) * 512],
                in_=xT[kc * 128 : (kc + 1) * 128, h * 512 : (h + 1) * 512],
            )
        for h in range(2):
            eng[(2 * kc + h + 1) % 2].dma_start(
                out=tv[:, h * 384 : (h + 1) * 384],
                in_=wqkvT[kc * 128 : (kc + 1) * 128, 2 * C + h * 384 : 2 * C + (h + 1) * 384],
            )
        xTs.append(tx)
        wvs.append(tv)
    bpb = persist.tile([128, C], F32, tag="bpb")
    nc.gpsimd.dma_start(
        out=bpb,
        in_=bass.AP(tensor=bproj.tensor, offset=bproj.offset, ap=[[0, 128]] + list(bproj.ap)),
    )

    def emit_wp_loads():
        # wproj is only needed by phase E; load it mid-kernel, off the
        # startup-critical DMA window.
        wps = []
        for kc in range(KC):
            t = persist.tile([128, C], F32R, tag=f"wp{kc}", name=f"wp{kc}")
            for h in range(2):
                eng[(2 * kc + h) % 2].dma_start(
                    out=t[:, h * 384 : (h + 1) * 384],
                    in_=wprojT[kc * 128 : (kc + 1) * 128, h * 384 : (h + 1) * 384],
                )
            wps.append(t)
        return wps

    # ---- phase B: V GEMM (natural layout, head-strided with ones column) ----
    v2s = []
    for nt in range(NT):
        v2s.append(persist.tile([128, H * (HD + 1)], BF16, tag=f"v2{nt}", name=f"v2{nt}"))
    for nt in range(NT):
        psv = ps_big.tile([128, 1024], F32, tag="ps_big")
        for kc in range(KC):
            lhsT = xTs[kc][:, nt * 128 : (nt + 1) * 128]
            nc.tensor.matmul(
                psv[:, 0:512], lhsT, wvs[kc][:, 0:512], start=(kc == 0), stop=(kc == KC - 1)
            )
            nc.tensor.matmul(
                psv[:, 512:768], lhsT, wvs[kc][:, 512:768], start=(kc == 0), stop=(kc == KC - 1)
            )
        v2v = v2s[nt].rearrange("p (h e) -> p h e", e=HD + 1)
        nc.vector.tensor_copy(
            out=v2v[:, :, 0:HD], in_=psv[:, 0:768].rearrange("p (h e) -> p h e", e=HD)
        )
        nc.vector.memset(v2v[:, :, HD : HD + 1], 1.0)

    # ---- phases C (qk GEMM) + D (attention), interleaved per head pair ----
    def emit_qk(j, qT, kT):
        # k first, then q: the k casts overlap the q GEMM, so attention's
        # first S matmul only waits on q's first half-cast.
        for dst, dcol0 in ((kT, C + j * 128), (qT, j * 128)):
            psq = ps_big.tile([128, 1024], F32, tag="ps_big", name="psqk")
            for kc in range(KC):
                w = wqk_pool.tile([128, 128], F32R, tag="wqk")
                nc.sync.dma_start(
                    out=w, in_=wqkvT[kc * 128 : (kc + 1) * 128, dcol0 : dcol0 + 128]
                )
                for h2 in range(NQT):
                    nc.tensor.matmul(
                        psq[:, h2 * 512 : (h2 + 1) * 512],
                        w,
                        xTs[kc][:, h2 * 512 : (h2 + 1) * 512],
                        start=(kc == 0),
                        stop=(kc == KC - 1),
                    )
            for h2 in range(NQT):
                nc.vector.tensor_copy(
                    out=dst[:, h2 * 512 : (h2 + 1) * 512], in_=psq[:, h2 * 512 : (h2 + 1) * 512]
                )

    def emit_attn(j, qT, kT, aT):
        # One [128,1024] S tile per nk chunk: head A in cols 0:512, head B in
        # 512:1024 — a single EXP covers both heads. Chunk-granular skew-2
        # software pipeline streaming continuously across the two nq tiles,
        # so the in-order PE never waits on ACT except at fill/drain.
        SKEW = 3
        steps = []
        oab = {}
        for step in range(NQT * NT + SKEW):
            if step < NQT * NT:
                nq, nkc = divmod(step, NT)
                s = ps_big.tile([128, 1024], F32, tag="ps_big", name="sAB")
                for half, kt0 in ((0, 0), (1, 64)):
                    nc.tensor.matmul(
                        s[:, half * 512 : (half + 1) * 512],
                        kT[kt0 : kt0 + 64, nkc * 128 : (nkc + 1) * 128],
                        qT[kt0 : kt0 + 64, nq * 512 : (nq + 1) * 512],
                        tile_position=(kt0, 0),
                    )
                e = expp.tile([128, 1024], BF16, tag="expp", name="eAB")
                nc.scalar.activation(out=e, in_=s, func=EXP, scale=SCALE)
                steps.append((nq, nkc, e))
            if step >= SKEW:
                nq, nkc, e = steps[step - SKEW]
                if nkc == 0:
                    oab[nq] = (
                        ps_o.tile([HD + 1, 512], F32, tag="ps_o", name="oA"),
                        ps_o.tile([HD + 1, 512], F32, tag="ps_o", name="oB"),
                    )
                oA, oB = oab[nq]
                v2v = v2s[nkc].rearrange("p (h e) -> p h e", e=HD + 1)
                nc.tensor.matmul(
                    oA,
                    v2v[:, 2 * j, :],
                    e[:, 0:512],
                    start=(nkc == 0),
                    stop=(nkc == NT - 1),
                )
                nc.tensor.matmul(
                    oB,
                    v2v[:, 2 * j + 1, :],
                    e[:, 512:1024],
                    start=(nkc == 0),
                    stop=(nkc == NT - 1),
                )
                if nkc == NT - 1:
                    for o, half in ((oA, 0), (oB, 1)):
                        # Drain O' to SBUF at once so the PSUM bank frees for
                        # the next nq tile; kick off the DRAM bounce that
                        # spreads the 512 denominators across 128 partitions.
                        # The reciprocal + multiply are emitted one pair later
                        # (emit_normalize) so their DMA dependencies are long
                        # resolved and never stall the DVE.
                        oc = rdp.tile([HD + 1, 512], F32, tag="oc")
                        nc.vector.tensor_copy(out=oc, in_=o)
                        scr = dram_scr.tile([1, 512], F32, tag="scr")
                        nc.gpsimd.dma_start(out=scr, in_=oc[HD : HD + 1, :])
                        rs = rdp.tile([128, 4], F32, tag="rs")
                        nc.gpsimd.dma_start(
                            out=rs,
                            in_=bass.AP(
                                tensor=scr.tensor, offset=scr.offset, ap=[[4, 128], [1, 4]]
                            ),
                        )
                        pending.append((oc, rs, aT, half, nq))

    def emit_normalize():
        for oc, rs, aT, half, nq in pending:
            rs2 = rdp.tile([128, 4], F32, tag="rs2")
            nc.vector.reciprocal(out=rs2, in_=rs)
            scr2 = dram_scr.tile([1, 512], F32, tag="scr2")
            nc.gpsimd.dma_start(
                out=bass.AP(tensor=scr2.tensor, offset=scr2.offset, ap=[[4, 128], [1, 4]]),
                in_=rs2,
            )
            rb = rdp.tile([64, 512], F32, tag="rb")
            nc.gpsimd.dma_start(
                out=rb,
                in_=bass.AP(tensor=scr2.tensor, offset=scr2.offset, ap=[[0, 64]] + list(scr2.ap[1:])),
            )
            nc.vector.tensor_mul(
                out=aT[half * 64 : half * 64 + 64, nq * 512 : (nq + 1) * 512],
                in0=oc[0:HD, :],
                in1=rb,
            )
        pending.clear()

    aTs = []
    qkts = []
    pending = []
    for j in range(PAIRS):
        qkts.append(
            (
                persist.tile([128, N], BF16, tag=f"qT{j}", name=f"qT{j}"),
                persist.tile([128, N], BF16, tag=f"kT{j}", name=f"kT{j}"),
            )
        )
    emit_qk(0, *qkts[0])
    for j in range(PAIRS):
        # one-pair lookahead: pair j+1's qk GEMM runs before pair j's
        # attention, so the qT/kT casts are always long done when the S
        # matmuls need them.
        if j + 1 < PAIRS:
            emit_qk(j + 1, *qkts[j + 1])
        aT = persist.tile([128, N], F32R, tag=f"aT{j}", name=f"aT{j}")
        aTs.append(aT)
        emit_normalize()  # flush pair j-1 (its bounce DMAs resolved long ago)
        emit_attn(j, *qkts[j], aT)
        if j == 1:
            wps = emit_wp_loads()
    emit_normalize()

    # ---- phase E: proj + bias ----
    for nt in range(NT):
        psy = ps_big.tile([128, 1024], F32, tag="ps_big", name="psy")
        for kc in range(KC):
            lhsT = aTs[kc][:, nt * 128 : (nt + 1) * 128]
            nc.tensor.matmul(
                psy[:, 0:512], lhsT, wps[kc][:, 0:512], start=(kc == 0), stop=(kc == KC - 1)
            )
            nc.tensor.matmul(
                psy[:, 512:768], lhsT, wps[kc][:, 512:768], start=(kc == 0), stop=(kc == KC - 1)
            )
        yb = work.tile([128, C], F32, tag="yb")
        nc.vector.tensor_add(out=yb, in0=psy[:, 0:768], in1=bpb)
        for h in range(2):
            eng[h].dma_start(
                out=y[nt * 128 : (nt + 1) * 128, h * 384 : (h + 1) * 384],
                in_=yb[:, h * 384 : (h + 1) * 384],
            )


def build():
    from contextlib import ExitStack

    nc = bacc.Bacc("TRN2", target_bir_lowering=False, debug=False)
    xT = nc.dram_tensor("xT", [C, N], F32R, kind="ExternalInput").ap()
    wqkvT = nc.dram_tensor("wqkvT", [C, 3 * C], F32R, kind="ExternalInput").ap()
    wprojT = nc.dram_tensor("wprojT", [C, C], F32R, kind="ExternalInput").ap()
    bproj = nc.dram_tensor("bproj", [C], F32, kind="ExternalInput").ap()
    y = nc.dram_tensor("y", [N, C], F32, kind="ExternalOutput").ap()
    with tile.TileContext(nc) as tc:
        with ExitStack() as ctx:
            _emit(tc, nc, xT, wqkvT, wprojT, bproj, y, ctx)
    nc.compile()
    return nc


_NC_CACHE = {}


def make_in_maps(x, w_qkv, w_proj, b_proj):
    wqkvT = np.ascontiguousarray(np.asarray(w_qkv).T)
    wprojT = np.ascontiguousarray(np.asarray(w_proj).T)
    b_proj = np.asarray(b_proj)
    return [
        {
            "xT": np.ascontiguousarray(np.asarray(x[b]).T),
            "wqkvT": wqkvT,
            "wprojT": wprojT,
            "bproj": b_proj,
        }
        for b in range(NCORES)
    ]


def kernel(x, w_qkv, w_proj, b_proj, _trace=False, _tmpdir=None):
    if "nc" not in _NC_CACHE:
        _NC_CACHE["nc"] = build()
    nc = _NC_CACHE["nc"]
    in_maps = make_in_maps(x, w_qkv, w_proj, b_proj)
    kwargs = {}
    if _trace:
        kwargs = {"trace": True, "tmpdir": _tmpdir}
    res = run_bass_kernel_spmd(nc, in_maps, core_ids=list(range(NCORES)), **kwargs)
    out = np.stack([res.results[i]["y"] for i in range(NCORES)], axis=0)
    if _trace:
        _NC_CACHE["last_result"] = res
    return out


if __name__ == "__main__":
    rng = np.random.default_rng(0)
    x = rng.standard_normal((B, N, C), dtype=np.float32)
    w_qkv = (rng.standard_normal((3 * C, C), dtype=np.float32) * C**-0.5).astype(np.float32)
    w_proj = (rng.standard_normal((C, C), dtype=np.float32) * C**-0.5).astype(np.float32)
    b_proj = np.zeros(C, dtype=np.float32)
    out = kernel(x, w_qkv, w_proj, b_proj)
    print("out", out.shape, out.dtype, float(np.abs(out).mean()))
